# revision 16
# baseline (speedup 1.0000x reference)
"""Spatial self-attention scores kernel for Trainium2 (8 NeuronCores).

Computes, per batch b:
    qk = W @ x_b          # [256, 4096] = [256,256] @ [256,4096]
    q, k = qk[:128], qk[128:]
    sim = (q.T @ k) * 128**-0.5
    out_b = softmax(sim, axis=-1)        # [4096, 4096]
Output: [8, 1, 4096, 4096] float32.

Sharding: data-parallel over batch, one batch image per NeuronCore.

The kernel is HBM-write-bound: 64 MiB of output per core vs ~29 us of
warm compute.  Everything is organized to (a) get the first output DMA
out as early as possible and (b) keep the output queue (sync HWDGE
ring) gapless at the HBM arbitration rate thereafter:

  - x is DMA'd in as fp32 bits re-typed to float32r (bitcast; HWDGE,
    no Q7 descriptor-generation bottleneck), four 1 MiB chunks split
    across BOTH HWDGE rings (scalar + sync) so the two queues drain in
    parallel.  W rides the otherwise-idle gpsimd (SWDGE) queue.
  - The projection runs on the PE straight from the fp32r x (2 cyc/row)
    -- no separate fp16 staging cast.  q/k are evicted from PSUM as
    fp16, so the 4096x4096 attention matmuls stream at 1 cyc/row: even
    a HAM-throttled (cold) PE sustains a group faster than the DMA
    drains it, so the PE can never gate the output queue.
  - Warm-up matmuls bracket the W-transpose so the PE reaches 2.4 GHz
    right as the first x chunk lands.
  - Row-tile 0 runs a fine-grained path: per-1024-column sim pieces and
    exps chase the arriving x chunks, so the first output bytes leave
    ~1.5 us after the last x chunk is projected.
  - per 128-query row-tile: 8 fp16 matmuls (K=128, N=512) into 4-bank
    PSUM tiles; one ScalarE ACTIVATE per 2048 columns computes
    exp(SCALE*sim) with a fused row-sum (accum_out); DVE combines the
    partial sums, takes the reciprocal, and scales the row.
  - output rows leave in 4 MB DMAs (two row-tiles per transfer; the
    first group ships per normalized half-row).
"""

import numpy as np
from contextlib import ExitStack

import concourse.bass as bass
import concourse.tile as tile
from concourse import bacc, mybir
from concourse.bass_utils import run_bass_kernel_spmd
from concourse.masks import make_identity

B = 8
C = 256
HW = 4096
D = 128
SCALE = D ** -0.5
N_CORES = 8

BANK = 512             # PSUM bank width (fp32) = one matmul free-dim
ACT_CHUNK = 2048       # one ScalarE activation spans 4 banks
N_ACT = HW // ACT_CHUNK          # 2
GRP = 2                # row-tiles per output DMA (2 -> 4 MB transfers)
N_GRP = HW // (128 * GRP)        # 16
OUT_BUFS = 4
X_CHUNK = 1024         # x input DMA granularity (4 chunks, 2 per ring)
N_XCHUNK = HW // X_CHUNK         # 4

F32 = mybir.dt.float32
# x lands in SBUF as float32r via a bitcast on the DRAM AP: the HWDGE
# DMA moves the same bits but the tile is *produced* as fp32r, so the
# projection can stream it through the PE at 2 cycles/row.
F32R = mybir.dt.float32r
# q/k live as fp16: the attention matmuls stream at 1 cycle/row and the
# 10-bit mantissa keeps the extra logit noise ~1e-3 (well inside the
# 2e-2 gate; measured end-to-end ~1e-3 scale-relative).
QK_DT = mybir.dt.float16
NWARM_A = 2            # warm-up matmuls before the W transpose
NWARM_B = 14           # ... and after, bridging the x-load window so the
                       # HAM clock gate stays at 2.4 GHz for the projection


def _emit(ctx: ExitStack, tc: tile.TileContext, out_ap, x_ap, w_ap):
    nc = tc.nc

    const = ctx.enter_context(tc.tile_pool(name="const", bufs=1))
    data = ctx.enter_context(tc.tile_pool(name="data", bufs=1))
    psum = ctx.enter_context(tc.tile_pool(name="psum", bufs=2, space="PSUM"))
    small = ctx.enter_context(tc.tile_pool(name="small", bufs=4))

    warm_f32 = const.tile([128, BANK], F32)
    nc.vector.memset(warm_f32, 0.0)
    warm16 = const.tile([128, BANK], QK_DT)
    nc.vector.tensor_copy(out=warm16, in_=warm_f32)

    # ---- W [256, 256] first, on the sync HWDGE ring: it gates the
    # transpose (and with it the whole projection pipeline).  The
    # scalar ring can't move data until ~10.4 us (its DGE is busy with
    # the exp ACT_TABLE_LOAD), so W + the first two x chunks ride sync
    # while the back half of x goes to scalar as one 2 MiB transfer
    # timed to land right as the sync ring finishes chunk 1.
    w_sb = const.tile([128, 2, C], F32)
    nc.sync.dma_start(out=w_sb, in_=w_ap.rearrange("(t p) c -> p t c", p=128))

    x_view = x_ap.bitcast(F32R).rearrange(
        "(t p) (c s) -> p c t s", p=128, s=X_CHUNK
    )
    x32 = data.tile([128, N_XCHUNK, 2, X_CHUNK], F32R)
    nc.sync.dma_start(out=x32[:, 0], in_=x_view[:, 0])
    nc.scalar.dma_start(out=x32[:, 1], in_=x_view[:, 1])
    nc.sync.dma_start(out=x32[:, 2], in_=x_view[:, 2])
    nc.scalar.dma_start(out=x32[:, 3], in_=x_view[:, 3])

    ident = const.tile([128, 128], F32)
    make_identity(nc, ident)

    # pull the exp table load off the first real activation
    tbl = small.tile([128, 1], F32, tag="tbl")
    nc.scalar.activation(
        out=tbl, in_=warm_f32[:, 0:1], func=mybir.ActivationFunctionType.Exp
    )

    # ---- PE warm-up: the HAM clock gate releases to 2.4 GHz only after
    # ~3.4 us of sustained activity; these throwaway matmuls (bracketing
    # the W transpose) make the projection and first sims run warm.
    wps = psum.tile([128, ACT_CHUNK], F32, tag="ps")
    for _ in range(NWARM_A):
        nc.tensor.matmul(
            wps[:, 0:BANK], warm16[:, 0:128], warm16, start=True, stop=True
        )

    # ---- transpose W on PE -> wt_sb[c_sub, c_tile, o] (contraction c on
    # partitions), evicted as fp32r for the 2-cyc/row projection.
    wt_sb = const.tile([128, 2, 2 * D], F32R)
    for t in range(2):          # output-channel tile (q half / k half)
        for ct in range(2):     # input-channel tile
            ps = psum.tile([128, ACT_CHUNK], F32, tag="ps")
            nc.tensor.transpose(
                ps[:, 0:128], w_sb[:, t, ct * 128:(ct + 1) * 128], ident
            )
            nc.vector.tensor_copy(
                out=wt_sb[:, ct, t * 128:(t + 1) * 128], in_=ps[:, 0:128]
            )

    wps2 = psum.tile([128, ACT_CHUNK], F32, tag="ps")
    for _ in range(NWARM_B):
        nc.tensor.matmul(
            wps2[:, 0:BANK], warm16[:, 0:128], warm16, start=True, stop=True
        )

    q_sb = data.tile([128, HW], QK_DT)
    k_sb = data.tile([128, HW], QK_DT)

    def proj_chunk(t, dst, c, banks=(0, 1), evict_split=False):
        """Project output-channel half t (0=q, 1=k) for x chunk c
        (columns [c*X_CHUNK, (c+1)*X_CHUNK)); banks selects the two
        512-wide banks within the chunk.  With evict_split, the fp16
        PSUM evictions alternate between the (startup-idle) ScalarE and
        the DVE so they don't serialize on one engine."""
        ps = psum.tile([128, ACT_CHUNK], F32, tag="ps")
        for jj in banks:
            lo = jj * BANK
            nc.tensor.matmul(
                ps[:, lo:lo + BANK], wt_sb[:, 0, t * 128:(t + 1) * 128],
                x32[:, c, 0, lo:lo + BANK], start=True, stop=False,
            )
            nc.tensor.matmul(
                ps[:, lo:lo + BANK], wt_sb[:, 1, t * 128:(t + 1) * 128],
                x32[:, c, 1, lo:lo + BANK], start=False, stop=True,
            )
        if evict_split and len(banks) == 2:
            # one wide eviction per chunk: fewer DVE instructions on the
            # startup critical path
            lo = banks[0] * BANK
            sl = slice(c * X_CHUNK + lo, c * X_CHUNK + lo + 2 * BANK)
            nc.vector.tensor_copy(out=dst[:, sl], in_=ps[:, lo:lo + 2 * BANK])
        else:
            for jj in banks:
                lo = jj * BANK
                sl = slice(c * X_CHUNK + lo, c * X_CHUNK + lo + BANK)
                nc.vector.tensor_copy(out=dst[:, sl], in_=ps[:, lo:lo + BANK])

    outp = None
    out_view = out_ap.rearrange("(g t p) m -> g p t m", t=GRP, p=128)

    def sim_chunk(lhs, out_row, a, accum):
        """One 2048-wide slice of one attention row: matmuls + fused exp."""
        ps = psum.tile([128, ACT_CHUNK], F32, tag="ps")
        for jj in range(ACT_CHUNK // BANK):
            sl = slice(a * ACT_CHUNK + jj * BANK, a * ACT_CHUNK + (jj + 1) * BANK)
            nc.tensor.matmul(
                ps[:, jj * BANK:(jj + 1) * BANK], lhs, k_sb[:, sl],
                start=True, stop=True,
            )
        nc.scalar.activation(
            out=out_row[:, a * ACT_CHUNK:(a + 1) * ACT_CHUNK],
            in_=ps[:, 0:ACT_CHUNK],
            func=mybir.ActivationFunctionType.Exp,
            scale=SCALE,
            accum_out=accum,
        )

    def norm_and_ship(out_grp, t, g, sums, split_dma):
        rsum = small.tile([128, 1], F32, tag="rsum")
        nc.vector.tensor_reduce(
            out=rsum, in_=sums, axis=mybir.AxisListType.X,
            op=mybir.AluOpType.add,
        )
        recip = small.tile([128, 1], F32, tag="recip")
        nc.vector.reciprocal(out=recip, in_=rsum)
        if split_dma:
            # normalize and ship each half-row as soon as it is scaled
            # (1 MB transfers) so the first outputs leave immediately
            i = g * GRP + t
            for a in range(N_ACT):
                sl = slice(a * ACT_CHUNK, (a + 1) * ACT_CHUNK)
                nc.vector.tensor_scalar_mul(
                    out=out_grp[:, t, sl], in0=out_grp[:, t, sl],
                    scalar1=recip,
                )
                nc.sync.dma_start(
                    out=out_ap[i * 128:(i + 1) * 128, sl],
                    in_=out_grp[:, t, sl],
                )
        else:
            nc.vector.tensor_scalar_mul(
                out=out_grp[:, t, :], in0=out_grp[:, t, :], scalar1=recip
            )

    def emit_group(g, split_dma=False):
        out_grp = outp.tile([128, GRP, HW], F32, tag="out")
        for t in range(GRP):
            lhs = q_sb[:, (g * GRP + t) * 128:(g * GRP + t + 1) * 128]
            sums = small.tile([128, N_ACT], F32, tag="sums")
            for a in range(N_ACT):
                sim_chunk(lhs, out_grp[:, t], a, sums[:, a:a + 1])
            norm_and_ship(out_grp, t, g, sums, split_dma)
        if not split_dma:
            nc.sync.dma_start(out=out_view[g], in_=out_grp)

    # ---- projection, chasing the x chunks in their arrival order
    # (chunk 1 completes first: it heads the scalar ring while chunk 0
    # queues behind W on sync).  Evictions split across ScalarE/DVE.
    proj_chunk(1, k_sb, 1, evict_split=True)   # k cols 1024:2048
    proj_chunk(1, k_sb, 0, evict_split=True)   # k cols    0:1024
    proj_chunk(0, q_sb, 0, banks=(0,), evict_split=True)  # q rows 0:512
    proj_chunk(1, k_sb, 2, evict_split=True)   # k cols 2048:3072
    proj_chunk(1, k_sb, 3, evict_split=True)   # k cols 3072:4096

    outp = ctx.enter_context(tc.tile_pool(name="outp", bufs=OUT_BUFS))

    # remaining q projections trickle in one 512-wide bank at a time,
    # each just ahead of the first group that reads it.  Groups 0-2
    # ship per half-row so the output queue never starves while the
    # ScalarE exp pipeline is ramping.
    emit_group(0, split_dma=True)
    emit_group(1, split_dma=True)
    for g in range(2, N_GRP):
        if g % 2 == 0:
            b = g // 2
            proj_chunk(0, q_sb, b // 2, banks=(b % 2,))
        emit_group(g, split_dma=(g == 2))


_built = None


def _get_nc():
    global _built
    if _built is None:
        nc = bacc.Bacc("TRN2", target_bir_lowering=False, debug=False)
        x = nc.dram_tensor("x", [C, HW], F32, kind="ExternalInput").ap()
        w = nc.dram_tensor("w", [2 * D, C], F32, kind="ExternalInput").ap()
        out = nc.dram_tensor("out", [HW, HW], F32, kind="ExternalOutput").ap()
        with tile.TileContext(nc) as tc:
            with ExitStack() as ctx:
                _emit(ctx, tc, out, x, w)
        nc.compile()
        _built = nc
    return _built


def kernel(x: np.ndarray, W: np.ndarray) -> np.ndarray:
    nc = _get_nc()
    x = np.asarray(x, dtype=np.float32)
    W = np.ascontiguousarray(np.asarray(W, dtype=np.float32))
    in_maps = [
        {"x": np.ascontiguousarray(x[b].reshape(C, HW)), "w": W} for b in range(B)
    ]
    res = run_bass_kernel_spmd(nc, in_maps, core_ids=list(range(N_CORES)))
    out = np.stack([res.results[b]["out"] for b in range(B)])
    return out[:, None]


# revision 19
# speedup vs baseline: 1.0434x; 1.0434x over previous
"""Spatial self-attention scores kernel for Trainium2 (8 NeuronCores).

Computes, per batch b:
    qk = W @ x_b          # [256, 4096] = [256,256] @ [256,4096]
    q, k = qk[:128], qk[128:]
    sim = (q.T @ k) * 128**-0.5
    out_b = softmax(sim, axis=-1)        # [4096, 4096]
Output: [8, 1, 4096, 4096] float32.

Sharding: data-parallel over batch, one batch image per NeuronCore.

The kernel is HBM-write-bound: 64 MiB of output per core vs ~29 us of
warm compute.  Everything is organized to (a) get the first output DMA
out as early as possible and (b) keep the output queue (sync HWDGE
ring) gapless at the HBM arbitration rate thereafter:

  - x is DMA'd in as fp32 bits re-typed to float32r (bitcast; HWDGE,
    no Q7 descriptor-generation bottleneck), four 1 MiB chunks split
    across BOTH HWDGE rings.  W heads the sync ring (it gates the
    transpose); the scalar ring -- whose DGE is busy with the exp
    ACT_TABLE_LOAD until ~10 us -- gets chunks that are needed later.
  - The projection runs on the PE straight from the fp32r x (2 cyc/row)
    -- no separate fp16 staging cast.  q/k are evicted from PSUM as
    fp16, so the 4096x4096 attention matmuls stream at 1 cyc/row: even
    a HAM-throttled (cold) PE sustains a group faster than the DMA
    drains it, so the PE can never gate the output queue.  (fp16 sims
    also speed up the ScalarE exps: less PSUM port contention.)
  - Warm-up matmuls bracket the W-transpose so the PE reaches 2.4 GHz
    right as the first x chunk lands; projections chase the chunks in
    arrival order, with PSUM evictions split across ScalarE and DVE.
  - per 128-query row-tile: 8 fp16 matmuls (K=128, N=512) into 4-bank
    PSUM tiles; one ScalarE ACTIVATE per 2048 columns computes
    exp(SCALE*sim) with a fused row-sum (accum_out); DVE combines the
    partial sums, takes the reciprocal, and scales the row.
  - output rows leave in 4 MB DMAs (two row-tiles per transfer); the
    first three groups ship per normalized half-row (1 MB) so the
    output queue never starves while the exp pipeline ramps.
"""

import numpy as np
from contextlib import ExitStack

import concourse.bass as bass
import concourse.tile as tile
from concourse import bacc, mybir
from concourse.bass_utils import run_bass_kernel_spmd
from concourse.masks import make_identity

B = 8
C = 256
HW = 4096
D = 128
SCALE = D ** -0.5
N_CORES = 8

BANK = 512             # PSUM bank width (fp32) = one matmul free-dim
ACT_CHUNK = 2048       # one ScalarE activation spans 4 banks
N_ACT = HW // ACT_CHUNK          # 2
GRP = 2                # row-tiles per output DMA (2 -> 4 MB transfers)
N_GRP = HW // (128 * GRP)        # 16
OUT_BUFS = 4
X_CHUNK = 1024         # x input DMA granularity (4 chunks, 2 per ring)
N_XCHUNK = HW // X_CHUNK         # 4

F32 = mybir.dt.float32
# x lands in SBUF as float32r via a bitcast on the DRAM AP: the HWDGE
# DMA moves the same bits but the tile is *produced* as fp32r, so the
# projection can stream it through the PE at 2 cycles/row.
F32R = mybir.dt.float32r
# q/k live as fp16: the attention matmuls stream at 1 cycle/row and the
# 10-bit mantissa keeps the extra logit noise ~1e-3 (well inside the
# 2e-2 gate; measured end-to-end ~1e-3 scale-relative).
QK_DT = mybir.dt.float16
NWARM_A = 2            # warm-up matmuls before the W transpose
NWARM_B = 6            # ... and after, spanning until x chunk 0 lands


def _emit(ctx: ExitStack, tc: tile.TileContext, out_ap, x_ap, w_ap):
    nc = tc.nc

    const = ctx.enter_context(tc.tile_pool(name="const", bufs=1))
    data = ctx.enter_context(tc.tile_pool(name="data", bufs=1))
    psum = ctx.enter_context(tc.tile_pool(name="psum", bufs=2, space="PSUM"))
    small = ctx.enter_context(tc.tile_pool(name="small", bufs=4))

    warm_f32 = const.tile([128, BANK], F32)
    nc.vector.memset(warm_f32, 0.0)
    warm16 = const.tile([128, BANK], QK_DT)
    nc.vector.tensor_copy(out=warm16, in_=warm_f32)

    # ---- W [256, 256] first, on the sync HWDGE ring: it gates the
    # transpose (and with it the whole projection pipeline).  The
    # scalar ring can't move data until ~10.4 us (its DGE is busy with
    # the exp ACT_TABLE_LOAD), so W + the first two x chunks ride sync
    # while the back half of x goes to scalar as one 2 MiB transfer
    # timed to land right as the sync ring finishes chunk 1.
    w_sb = const.tile([128, 2, C], F32)
    nc.sync.dma_start(out=w_sb, in_=w_ap.rearrange("(t p) c -> p t c", p=128))

    x_view = x_ap.bitcast(F32R).rearrange(
        "(t p) (c s) -> p c t s", p=128, s=X_CHUNK
    )
    x32 = data.tile([128, N_XCHUNK, 2, X_CHUNK], F32R)
    nc.sync.dma_start(out=x32[:, 0], in_=x_view[:, 0])
    nc.scalar.dma_start(out=x32[:, 1], in_=x_view[:, 1])
    nc.sync.dma_start(out=x32[:, 2], in_=x_view[:, 2])
    nc.scalar.dma_start(out=x32[:, 3], in_=x_view[:, 3])

    ident = const.tile([128, 128], F32)
    make_identity(nc, ident)

    # pull the exp table load off the first real activation
    tbl = small.tile([128, 1], F32, tag="tbl")
    nc.scalar.activation(
        out=tbl, in_=warm_f32[:, 0:1], func=mybir.ActivationFunctionType.Exp
    )

    # ---- PE warm-up: the HAM clock gate releases to 2.4 GHz only after
    # ~3.4 us of sustained activity; these throwaway matmuls (bracketing
    # the W transpose) make the projection and first sims run warm.
    wps = psum.tile([128, ACT_CHUNK], F32, tag="ps")
    for _ in range(NWARM_A):
        nc.tensor.matmul(
            wps[:, 0:BANK], warm16[:, 0:128], warm16, start=True, stop=True
        )

    # ---- transpose W on PE -> wt_sb[c_sub, c_tile, o] (contraction c on
    # partitions), evicted as fp32r for the 2-cyc/row projection.
    wt_sb = const.tile([128, 2, 2 * D], F32R)
    for t in range(2):          # output-channel tile (q half / k half)
        for ct in range(2):     # input-channel tile
            ps = psum.tile([128, ACT_CHUNK], F32, tag="ps")
            nc.tensor.transpose(
                ps[:, 0:128], w_sb[:, t, ct * 128:(ct + 1) * 128], ident
            )
            nc.vector.tensor_copy(
                out=wt_sb[:, ct, t * 128:(t + 1) * 128], in_=ps[:, 0:128]
            )

    wps2 = psum.tile([128, ACT_CHUNK], F32, tag="ps")
    for _ in range(NWARM_B):
        nc.tensor.matmul(
            wps2[:, 0:BANK], warm16[:, 0:128], warm16, start=True, stop=True
        )

    q_sb = data.tile([128, HW], QK_DT)
    k_sb = data.tile([128, HW], QK_DT)

    def proj_chunk(t, dst, c, banks=(0, 1), evict_split=False):
        """Project output-channel half t (0=q, 1=k) for x chunk c
        (columns [c*X_CHUNK, (c+1)*X_CHUNK)); banks selects the two
        512-wide banks within the chunk.  With evict_split, the fp16
        PSUM evictions alternate between the (startup-idle) ScalarE and
        the DVE so they don't serialize on one engine."""
        ps = psum.tile([128, ACT_CHUNK], F32, tag="ps")
        for jj in banks:
            lo = jj * BANK
            nc.tensor.matmul(
                ps[:, lo:lo + BANK], wt_sb[:, 0, t * 128:(t + 1) * 128],
                x32[:, c, 0, lo:lo + BANK], start=True, stop=False,
            )
            nc.tensor.matmul(
                ps[:, lo:lo + BANK], wt_sb[:, 1, t * 128:(t + 1) * 128],
                x32[:, c, 1, lo:lo + BANK], start=False, stop=True,
            )
            sl = slice(c * X_CHUNK + lo, c * X_CHUNK + lo + BANK)
            if evict_split and jj == 0:
                nc.scalar.copy(out=dst[:, sl], in_=ps[:, lo:lo + BANK])
            else:
                nc.vector.tensor_copy(out=dst[:, sl], in_=ps[:, lo:lo + BANK])

    outp = None
    out_view = out_ap.rearrange("(g t p) m -> g p t m", t=GRP, p=128)

    def sim_chunk(lhs, out_row, a, accum):
        """One 2048-wide slice of one attention row: matmuls + fused exp."""
        ps = psum.tile([128, ACT_CHUNK], F32, tag="ps")
        for jj in range(ACT_CHUNK // BANK):
            sl = slice(a * ACT_CHUNK + jj * BANK, a * ACT_CHUNK + (jj + 1) * BANK)
            nc.tensor.matmul(
                ps[:, jj * BANK:(jj + 1) * BANK], lhs, k_sb[:, sl],
                start=True, stop=True,
            )
        nc.scalar.activation(
            out=out_row[:, a * ACT_CHUNK:(a + 1) * ACT_CHUNK],
            in_=ps[:, 0:ACT_CHUNK],
            func=mybir.ActivationFunctionType.Exp,
            scale=SCALE,
            accum_out=accum,
        )

    def norm_and_ship(out_grp, t, g, sums, split_dma):
        rsum = small.tile([128, 1], F32, tag="rsum")
        nc.vector.tensor_reduce(
            out=rsum, in_=sums, axis=mybir.AxisListType.X,
            op=mybir.AluOpType.add,
        )
        recip = small.tile([128, 1], F32, tag="recip")
        nc.vector.reciprocal(out=recip, in_=rsum)
        if split_dma:
            # normalize and ship each half-row as soon as it is scaled
            # (1 MB transfers) so the first outputs leave immediately
            i = g * GRP + t
            for a in range(N_ACT):
                sl = slice(a * ACT_CHUNK, (a + 1) * ACT_CHUNK)
                nc.vector.tensor_scalar_mul(
                    out=out_grp[:, t, sl], in0=out_grp[:, t, sl],
                    scalar1=recip,
                )
                nc.sync.dma_start(
                    out=out_ap[i * 128:(i + 1) * 128, sl],
                    in_=out_grp[:, t, sl],
                )
        else:
            nc.vector.tensor_scalar_mul(
                out=out_grp[:, t, :], in0=out_grp[:, t, :], scalar1=recip
            )

    def emit_group(g, split_dma=False):
        out_grp = outp.tile([128, GRP, HW], F32, tag="out")
        for t in range(GRP):
            lhs = q_sb[:, (g * GRP + t) * 128:(g * GRP + t + 1) * 128]
            sums = small.tile([128, N_ACT], F32, tag="sums")
            for a in range(N_ACT):
                sim_chunk(lhs, out_grp[:, t], a, sums[:, a:a + 1])
            norm_and_ship(out_grp, t, g, sums, split_dma)
        if not split_dma:
            nc.sync.dma_start(out=out_view[g], in_=out_grp)

    # ---- projection, chasing the x chunks in their arrival order
    # (chunk 1 completes first: it heads the scalar ring while chunk 0
    # queues behind W on sync).  Evictions split across ScalarE/DVE.
    proj_chunk(1, k_sb, 1, evict_split=True)   # k cols 1024:2048
    proj_chunk(1, k_sb, 0, evict_split=True)   # k cols    0:1024
    proj_chunk(0, q_sb, 0, banks=(0,), evict_split=True)  # q rows 0:512
    proj_chunk(1, k_sb, 2, evict_split=True)   # k cols 2048:3072
    proj_chunk(1, k_sb, 3, evict_split=True)   # k cols 3072:4096

    outp = ctx.enter_context(tc.tile_pool(name="outp", bufs=OUT_BUFS))

    # remaining q projections trickle in one 512-wide bank at a time,
    # each just ahead of the first group that reads it.  Groups 0-2
    # ship per half-row so the output queue never starves while the
    # ScalarE exp pipeline is ramping.
    emit_group(0, split_dma=True)
    emit_group(1, split_dma=True)
    for g in range(2, N_GRP):
        if g % 2 == 0:
            b = g // 2
            proj_chunk(0, q_sb, b // 2, banks=(b % 2,))
        emit_group(g, split_dma=(g == 2))


_built = None


def _get_nc():
    global _built
    if _built is None:
        nc = bacc.Bacc("TRN2", target_bir_lowering=False, debug=False)
        x = nc.dram_tensor("x", [C, HW], F32, kind="ExternalInput").ap()
        w = nc.dram_tensor("w", [2 * D, C], F32, kind="ExternalInput").ap()
        out = nc.dram_tensor("out", [HW, HW], F32, kind="ExternalOutput").ap()
        with tile.TileContext(nc) as tc:
            with ExitStack() as ctx:
                _emit(ctx, tc, out, x, w)
        nc.compile()
        _built = nc
    return _built


def kernel(x: np.ndarray, W: np.ndarray) -> np.ndarray:
    nc = _get_nc()
    x = np.asarray(x, dtype=np.float32)
    W = np.ascontiguousarray(np.asarray(W, dtype=np.float32))
    in_maps = [
        {"x": np.ascontiguousarray(x[b].reshape(C, HW)), "w": W} for b in range(B)
    ]
    res = run_bass_kernel_spmd(nc, in_maps, core_ids=list(range(N_CORES)))
    out = np.stack([res.results[b]["out"] for b in range(B)])
    return out[:, None]


# revision 23
# speedup vs baseline: 1.0481x; 1.0044x over previous
"""Spatial self-attention scores kernel for Trainium2 (8 NeuronCores).

Computes, per batch b:
    qk = W @ x_b          # [256, 4096] = [256,256] @ [256,4096]
    q, k = qk[:128], qk[128:]
    sim = (q.T @ k) * 128**-0.5
    out_b = softmax(sim, axis=-1)        # [4096, 4096]
Output: [8, 1, 4096, 4096] float32.

Sharding: data-parallel over batch, one batch image per NeuronCore.

The kernel is HBM-write-bound: 64 MiB of output per core vs ~29 us of
warm compute.  Everything is organized to (a) get the first output DMA
out as early as possible and (b) keep the output queue (sync HWDGE
ring) gapless at the HBM arbitration rate thereafter:

  - x is DMA'd in as fp32 bits re-typed to float32r (bitcast; HWDGE,
    no Q7 descriptor-generation bottleneck), four 1 MiB chunks split
    across BOTH HWDGE rings.  W heads the sync ring (it gates the
    transpose); the scalar ring -- whose DGE is busy with the exp
    ACT_TABLE_LOAD until ~10 us -- gets chunks that are needed later.
  - The projection runs on the PE straight from the fp32r x (2 cyc/row)
    -- no separate fp16 staging cast.  q/k are evicted from PSUM as
    fp16, so the 4096x4096 attention matmuls stream at 1 cyc/row: even
    a HAM-throttled (cold) PE sustains a group faster than the DMA
    drains it, so the PE can never gate the output queue.  (fp16 sims
    also speed up the ScalarE exps: less PSUM port contention.)
  - Warm-up matmuls bracket the W-transpose so the PE reaches 2.4 GHz
    right as the first x chunk lands; projections chase the chunks in
    arrival order, with PSUM evictions split across ScalarE and DVE.
  - per 128-query row-tile: 8 fp16 matmuls (K=128, N=512) into 4-bank
    PSUM tiles; one ScalarE ACTIVATE per 2048 columns computes
    exp(SCALE*sim) with a fused row-sum (accum_out); DVE combines the
    partial sums, takes the reciprocal, and scales the row.
  - output rows leave in 4 MB DMAs (two row-tiles per transfer); the
    first three groups ship per normalized half-row (1 MB) so the
    output queue never starves while the exp pipeline ramps.
"""

import numpy as np
from contextlib import ExitStack

import concourse.bass as bass
import concourse.tile as tile
from concourse import bacc, mybir
from concourse.bass_utils import run_bass_kernel_spmd
from concourse.masks import make_identity

B = 8
C = 256
HW = 4096
D = 128
SCALE = D ** -0.5
N_CORES = 8

BANK = 512             # PSUM bank width (fp32) = one matmul free-dim
ACT_CHUNK = 2048       # one ScalarE activation spans 4 banks
N_ACT = HW // ACT_CHUNK          # 2
GRP = 2                # row-tiles per output DMA (2 -> 4 MB transfers)
N_GRP = HW // (128 * GRP)        # 16
OUT_BUFS = 4
X_CHUNK = 1024         # x input DMA granularity (4 chunks, 2 per ring)
N_XCHUNK = HW // X_CHUNK         # 4

F32 = mybir.dt.float32
# x lands in SBUF as float32r via a bitcast on the DRAM AP: the HWDGE
# DMA moves the same bits but the tile is *produced* as fp32r, so the
# projection can stream it through the PE at 2 cycles/row.
F32R = mybir.dt.float32r
# q/k live as fp16: the attention matmuls stream at 1 cycle/row and the
# 10-bit mantissa keeps the extra logit noise ~1e-3 (well inside the
# 2e-2 gate; measured end-to-end ~1e-3 scale-relative).
QK_DT = mybir.dt.float16
NWARM_A = 2            # warm-up matmuls before the W transpose
NWARM_B = 6            # ... and after, spanning until x chunk 0 lands


def _emit(ctx: ExitStack, tc: tile.TileContext, out_ap, x_ap, w_ap):
    nc = tc.nc

    const = ctx.enter_context(tc.tile_pool(name="const", bufs=1))
    data = ctx.enter_context(tc.tile_pool(name="data", bufs=1))
    psum = ctx.enter_context(tc.tile_pool(name="psum", bufs=2, space="PSUM"))
    small = ctx.enter_context(tc.tile_pool(name="small", bufs=4))

    warm_f32 = const.tile([128, BANK], F32)
    nc.vector.memset(warm_f32, 0.0)
    warm16 = const.tile([128, BANK], QK_DT)
    nc.vector.tensor_copy(out=warm16, in_=warm_f32)

    # ---- W [256, 256] first, on the sync HWDGE ring: it gates the
    # transpose (and with it the whole projection pipeline).  The
    # scalar ring can't move data until ~10.4 us (its DGE is busy with
    # the exp ACT_TABLE_LOAD), so W + the first two x chunks ride sync
    # while the back half of x goes to scalar as one 2 MiB transfer
    # timed to land right as the sync ring finishes chunk 1.
    w_sb = const.tile([128, 2, C], F32)
    nc.sync.dma_start(out=w_sb, in_=w_ap.rearrange("(t p) c -> p t c", p=128))

    x_view = x_ap.bitcast(F32R).rearrange(
        "(t p) (c s) -> p c t s", p=128, s=X_CHUNK
    )
    x32 = data.tile([128, N_XCHUNK, 2, X_CHUNK], F32R)
    nc.sync.dma_start(out=x32[:, 0], in_=x_view[:, 0])
    nc.scalar.dma_start(out=x32[:, 1], in_=x_view[:, 1])
    nc.sync.dma_start(out=x32[:, 2], in_=x_view[:, 2])
    nc.scalar.dma_start(out=x32[:, 3], in_=x_view[:, 3])

    ident = const.tile([128, 128], F32)
    make_identity(nc, ident)

    # pull the exp table load off the first real activation
    tbl = small.tile([128, 1], F32, tag="tbl")
    nc.scalar.activation(
        out=tbl, in_=warm_f32[:, 0:1], func=mybir.ActivationFunctionType.Exp
    )

    # ---- PE warm-up: the HAM clock gate releases to 2.4 GHz only after
    # ~3.4 us of sustained activity; these throwaway matmuls (bracketing
    # the W transpose) make the projection and first sims run warm.
    wps = psum.tile([128, ACT_CHUNK], F32, tag="ps")
    for _ in range(NWARM_A):
        nc.tensor.matmul(
            wps[:, 0:BANK], warm16[:, 0:128], warm16, start=True, stop=True
        )

    # ---- transpose W on PE -> wt_sb[c_sub, c_tile, o] (contraction c on
    # partitions), evicted as fp32r for the 2-cyc/row projection.
    wt_sb = const.tile([128, 2, 2 * D], F32R)
    for t in range(2):          # output-channel tile (q half / k half)
        for ct in range(2):     # input-channel tile
            ps = psum.tile([128, ACT_CHUNK], F32, tag="ps")
            nc.tensor.transpose(
                ps[:, 0:128], w_sb[:, t, ct * 128:(ct + 1) * 128], ident
            )
            nc.vector.tensor_copy(
                out=wt_sb[:, ct, t * 128:(t + 1) * 128], in_=ps[:, 0:128]
            )

    wps2 = psum.tile([128, ACT_CHUNK], F32, tag="ps")
    for _ in range(NWARM_B):
        nc.tensor.matmul(
            wps2[:, 0:BANK], warm16[:, 0:128], warm16, start=True, stop=True
        )

    q_sb = data.tile([128, HW], QK_DT)
    k_sb = data.tile([128, HW], QK_DT)

    def proj_chunk(t, dst, c, banks=(0, 1), evict_split=False):
        """Project output-channel half t (0=q, 1=k) for x chunk c
        (columns [c*X_CHUNK, (c+1)*X_CHUNK)); banks selects the two
        512-wide banks within the chunk.  With evict_split, the fp16
        PSUM evictions alternate between the (startup-idle) ScalarE and
        the DVE so they don't serialize on one engine."""
        ps = psum.tile([128, ACT_CHUNK], F32, tag="ps")
        for jj in banks:
            lo = jj * BANK
            nc.tensor.matmul(
                ps[:, lo:lo + BANK], wt_sb[:, 0, t * 128:(t + 1) * 128],
                x32[:, c, 0, lo:lo + BANK], start=True, stop=False,
            )
            nc.tensor.matmul(
                ps[:, lo:lo + BANK], wt_sb[:, 1, t * 128:(t + 1) * 128],
                x32[:, c, 1, lo:lo + BANK], start=False, stop=True,
            )
            sl = slice(c * X_CHUNK + lo, c * X_CHUNK + lo + BANK)
            if evict_split and jj == 0:
                nc.scalar.copy(out=dst[:, sl], in_=ps[:, lo:lo + BANK])
            else:
                nc.vector.tensor_copy(out=dst[:, sl], in_=ps[:, lo:lo + BANK])

    outp = None
    out_view = out_ap.rearrange("(g t p) m -> g p t m", t=GRP, p=128)

    def sim_chunk(lhs, out_row, a, accum):
        """One 2048-wide slice of one attention row: matmuls + fused exp."""
        ps = psum.tile([128, ACT_CHUNK], F32, tag="ps")
        for jj in range(ACT_CHUNK // BANK):
            sl = slice(a * ACT_CHUNK + jj * BANK, a * ACT_CHUNK + (jj + 1) * BANK)
            nc.tensor.matmul(
                ps[:, jj * BANK:(jj + 1) * BANK], lhs, k_sb[:, sl],
                start=True, stop=True,
            )
        nc.scalar.activation(
            out=out_row[:, a * ACT_CHUNK:(a + 1) * ACT_CHUNK],
            in_=ps[:, 0:ACT_CHUNK],
            func=mybir.ActivationFunctionType.Exp,
            scale=SCALE,
            accum_out=accum,
        )

    def norm_and_ship(out_grp, t, g, sums, split_dma, fine=False):
        rsum = small.tile([128, 1], F32, tag="rsum")
        nc.vector.tensor_reduce(
            out=rsum, in_=sums, axis=mybir.AxisListType.X,
            op=mybir.AluOpType.add,
        )
        recip = small.tile([128, 1], F32, tag="recip")
        nc.vector.reciprocal(out=recip, in_=rsum)
        if split_dma:
            # normalize and ship each half-row as soon as it is scaled
            # (1 MB transfers) so the first outputs leave immediately;
            # the very first row-tile ships 1024-wide quarter-rows so
            # the first bytes leave one DVE-scale earlier still.
            i = g * GRP + t
            n_piece = 2 * N_ACT if fine else N_ACT
            piece = HW // n_piece
            for a in range(n_piece):
                sl = slice(a * piece, (a + 1) * piece)
                nc.vector.tensor_scalar_mul(
                    out=out_grp[:, t, sl], in0=out_grp[:, t, sl],
                    scalar1=recip,
                )
                nc.sync.dma_start(
                    out=out_ap[i * 128:(i + 1) * 128, sl],
                    in_=out_grp[:, t, sl],
                )
        else:
            nc.vector.tensor_scalar_mul(
                out=out_grp[:, t, :], in0=out_grp[:, t, :], scalar1=recip
            )

    def emit_group(g, split_dma=False, fine=False):
        out_grp = outp.tile([128, GRP, HW], F32, tag="out")
        for t in range(GRP):
            lhs = q_sb[:, (g * GRP + t) * 128:(g * GRP + t + 1) * 128]
            sums = small.tile([128, N_ACT], F32, tag="sums")
            for a in range(N_ACT):
                sim_chunk(lhs, out_grp[:, t], a, sums[:, a:a + 1])
            norm_and_ship(out_grp, t, g, sums, split_dma, fine and t == 0)
        if not split_dma:
            nc.sync.dma_start(out=out_view[g], in_=out_grp)

    # ---- projection, chasing the x chunks in their arrival order
    # (chunk 1 completes first: it heads the scalar ring while chunk 0
    # queues behind W on sync).  Evictions split across ScalarE/DVE.
    proj_chunk(1, k_sb, 1, evict_split=True)   # k cols 1024:2048
    proj_chunk(1, k_sb, 0, evict_split=True)   # k cols    0:1024
    # q bank 0 (rows 0:512): row-tile 0's 128 stationary columns are
    # evicted first as a small ScalarE copy, so the first sim unblocks
    # ~1 us before the rest of the bank lands via DVE.
    psq = psum.tile([128, ACT_CHUNK], F32, tag="ps")
    nc.tensor.matmul(
        psq[:, 0:BANK], wt_sb[:, 0, 0:128], x32[:, 0, 0, 0:BANK],
        start=True, stop=False,
    )
    nc.tensor.matmul(
        psq[:, 0:BANK], wt_sb[:, 1, 0:128], x32[:, 0, 1, 0:BANK],
        start=False, stop=True,
    )
    nc.scalar.copy(out=q_sb[:, 0:128], in_=psq[:, 0:128])
    nc.vector.tensor_copy(out=q_sb[:, 128:BANK], in_=psq[:, 128:BANK])
    proj_chunk(1, k_sb, 2, evict_split=True)   # k cols 2048:3072
    proj_chunk(1, k_sb, 3, evict_split=True)   # k cols 3072:4096

    outp = ctx.enter_context(tc.tile_pool(name="outp", bufs=OUT_BUFS))

    # remaining q projections trickle in one 512-wide bank at a time,
    # each just ahead of the first group that reads it.  Groups 0-2
    # ship per half-row so the output queue never starves while the
    # ScalarE exp pipeline is ramping.
    emit_group(0, split_dma=True, fine=True)
    emit_group(1, split_dma=True)
    for g in range(2, N_GRP):
        if g % 2 == 0:
            b = g // 2
            proj_chunk(0, q_sb, b // 2, banks=(b % 2,))
        emit_group(g, split_dma=(g == 2))


_built = None


def _get_nc():
    global _built
    if _built is None:
        nc = bacc.Bacc("TRN2", target_bir_lowering=False, debug=False)
        x = nc.dram_tensor("x", [C, HW], F32, kind="ExternalInput").ap()
        w = nc.dram_tensor("w", [2 * D, C], F32, kind="ExternalInput").ap()
        out = nc.dram_tensor("out", [HW, HW], F32, kind="ExternalOutput").ap()
        with tile.TileContext(nc) as tc:
            with ExitStack() as ctx:
                _emit(ctx, tc, out, x, w)
        nc.compile()
        _built = nc
    return _built


def kernel(x: np.ndarray, W: np.ndarray) -> np.ndarray:
    nc = _get_nc()
    x = np.asarray(x, dtype=np.float32)
    W = np.ascontiguousarray(np.asarray(W, dtype=np.float32))
    in_maps = [
        {"x": np.ascontiguousarray(x[b].reshape(C, HW)), "w": W} for b in range(B)
    ]
    res = run_bass_kernel_spmd(nc, in_maps, core_ids=list(range(N_CORES)))
    out = np.stack([res.results[b]["out"] for b in range(B)])
    return out[:, None]
